# revision 1
# baseline (speedup 1.0000x reference)
"""Trainium2 Bass kernel for nn_Memory_30571577213131 (scatter_memory).

Slot-memory module: T=3 recurrence steps of {LayerNorm -> write-MHA(mem, z, z)
-> GRUCell} followed by a read-MHA(z, mem, mem).

Sharding: pure data parallel - batch B=64 split as 8 batches per core across
8 NeuronCores; all parameters replicated.

Layout strategy (per core):
  - All activations kept feature-major ("X.T": partitions = feature chunk of
    128, free dim = tokens/slots), so every projection is a chain of 6
    PSUM-accumulated matmuls with N=512 moving columns.
  - 4 batches (4*128 slots = 512) are grouped in the free dim for all
    slot-side dense ops (LN / Q / Wo / GRU) so matmuls run at N=512.
  - fp16 operands everywhere on the PE (fp22 multiply, fp32 accumulate),
    fp32 I/O at the boundaries. Weights pre-transposed/cast on host.
  - Softmax row-wise via ACT Exp with fused accum (row sums); partition-dim
    softmax (read attention) via ones-matmul column sums + broadcast matmul.
  - V tensors round-trip through DRAM to keep SBUF under the 192KB/partition
    cap; K stays resident.
"""

import numpy as np
from contextlib import ExitStack

import concourse.bass as bass
import concourse.tile as tile
from concourse import bacc, mybir
from concourse import bass_utils
from concourse.masks import make_identity

f16 = mybir.dt.float16
f32 = mybir.dt.float32
f32r = mybir.dt.float32r
AF = mybir.ActivationFunctionType
Alu = mybir.AluOpType

P = 128
E = 768
EC = E // P          # 6 feature chunks
S = 128              # slots
T = 3                # recurrence steps
B = 64
L = 512
NCORE = 8
NB = B // NCORE      # 8 batches per core
GB = 4               # batches per group (4*128 slots = 512 free dim)
NG = NB // GB        # 2 groups
LN_EPS = 1e-5

# bias table column groups (each 6 wide) in the [128, 66] bias tile
BK, BQ, BO, BR, BZ, BIN, BHN, RBQ, RBK, LNG, LNB = range(11)

_CACHE = {}


def _emit(nc, tc, ctx, D):
    cp = ctx.enter_context(tc.tile_pool(name="consts", bufs=1))
    wp = ctx.enter_context(tc.tile_pool(name="wts", bufs=4))
    zp = ctx.enter_context(tc.tile_pool(name="ztp", bufs=2))
    kp = ctx.enter_context(tc.tile_pool(name="kvp", bufs=1))
    mp = ctx.enter_context(tc.tile_pool(name="memp", bufs=1))
    mnp = ctx.enter_context(tc.tile_pool(name="memn", bufs=2))
    bap = ctx.enter_context(tc.tile_pool(name="bigact", bufs=4))
    otp = ctx.enter_context(tc.tile_pool(name="otp", bufs=2))
    vtp = ctx.enter_context(tc.tile_pool(name="vtp", bufs=2))
    sp = ctx.enter_context(tc.tile_pool(name="scratch", bufs=3))
    sp3 = ctx.enter_context(tc.tile_pool(name="scratch3", bufs=3))
    ssp = ctx.enter_context(tc.tile_pool(name="small", bufs=2))
    op = ctx.enter_context(tc.tile_pool(name="outp", bufs=2))
    dp = ctx.enter_context(tc.tile_pool(name="vdram", bufs=1, space="DRAM"))
    psA = ctx.enter_context(tc.tile_pool(name="psA", bufs=4, space="PSUM"))
    psB = ctx.enter_context(tc.tile_pool(name="psB", bufs=2, space="PSUM"))
    psT = ctx.enter_context(tc.tile_pool(name="psT", bufs=2, space="PSUM"))

    # ---- constants
    idy = cp.tile([P, P], f16, tag="idy")
    make_identity(nc, idy[:])
    ones_c16 = cp.tile([P, 1], f16, tag="oc16")
    nc.vector.memset(ones_c16[:], 1.0)
    ones_r16 = cp.tile([1, P], f16, tag="or16")
    nc.vector.memset(ones_r16[:], 1.0)
    ones_c32 = cp.tile([P, 1], f32r, tag="oc32")
    nc.scalar.copy(ones_c32[:], ones_c16[:])
    ones_r32 = cp.tile([1, P], f32r, tag="or32")
    nc.scalar.copy(ones_r32[:], ones_r16[:])
    eps128 = cp.tile([P, 1], f32, tag="eps128")
    nc.vector.memset(eps128[:], LN_EPS)
    bias = cp.tile([P, 66], f32, tag="bias")
    nc.sync.dma_start(bias[:], D["bias"])
    brep16 = cp.tile([P, 3 * E], f16, tag="brep16")
    nc.sync.dma_start(brep16[:], D["brep16"])

    def bcol(i, c):
        return bias[:, i * 6 + c : i * 6 + c + 1]

    def load_w(name):
        w = wp.tile([P, EC, E], f16, tag="w")
        nc.sync.dma_start(w[:], D[name].rearrange("(c p) f -> p c f", p=P))
        return w

    # ---- memory init from slots (broadcast to every batch)
    mem = []
    for g in range(NG):
        m = mp.tile([P, EC, 512], f16, tag=f"mem{g}")
        for bi in range(GB):
            nc.sync.dma_start(
                m[:, :, bi * 128 : (bi + 1) * 128],
                D["slots"].rearrange("(c p) s -> p c s", p=P),
            )
        mem.append(m)

    # ---- phase 1: K = z@Wk.T + bk (feature-major), V = z@Wv.T + bv (token-major)
    wk = load_w("wk")
    wv = load_w("wv")
    K = []
    Vd = []
    for b in range(NB):
        zt = zp.tile([P, EC, L], f16, tag="zt")
        nc.sync.dma_start(zt[:], D["z"][b].rearrange("(c p) t -> p c t", p=P))
        kt = kp.tile([P, EC, L], f16, tag=f"k{b}")
        for of in range(EC):
            ps = psA.tile([P, L], f32, tag="psA")
            for e in range(EC):
                nc.tensor.matmul(
                    ps[:],
                    lhsT=wk[:, e, of * 128 : (of + 1) * 128],
                    rhs=zt[:, e, :],
                    start=(e == 0),
                    stop=(e == EC - 1),
                )
            nc.scalar.activation(kt[:, of, :], ps[:], AF.Identity, bias=bcol(BK, of))
        K.append(kt)
        vsb = vtp.tile([P, 4, E], f16, tag="vt")
        for t4 in range(4):
            for n0, nw in ((0, 512), (512, 256)):
                ps = psA.tile([P, nw], f32, tag="psA")
                for e in range(EC):
                    nc.tensor.matmul(
                        ps[:],
                        lhsT=zt[:, e, t4 * 128 : (t4 + 1) * 128],
                        rhs=wv[:, e, n0 : n0 + nw],
                        start=(e == 0),
                        stop=(e == EC - 1),
                    )
                nc.vector.tensor_add(
                    vsb[:, t4, n0 : n0 + nw], ps[:], brep16[:, n0 : n0 + nw]
                )
        vd = dp.tile([P, 4, E], f16, tag=f"vd{b}")
        nc.sync.dma_start(vd[:], vsb[:])
        Vd.append(vd)

    # ---- LayerNorm emitter: partition-dim sums via ones-matmuls, then all
    # scalar math on [128,512] broadcast tiles (full DVE/ACT lane width).
    def emit_ln(g):
        mn = mnp.tile([P, EC, 512], f16, tag="mn")
        psx = psA.tile([1, 512], f32, tag="psA")
        for e in range(EC):
            nc.tensor.matmul(
                psx[:], lhsT=ones_c16[:], rhs=mem[g][:, e, :],
                start=(e == 0), stop=(e == EC - 1),
            )
        psq = psA.tile([1, 512], f32, tag="psA")
        for e in range(EC):
            sq = sp.tile([P, 512], f32r, tag="t32")
            nc.scalar.square(sq[:], mem[g][:, e, :])
            nc.tensor.matmul(
                psq[:], lhsT=ones_c32[:], rhs=sq[:],
                start=(e == 0), stop=(e == EC - 1),
            )
        sxr = ssp.tile([1, 512], f32r, tag="sxr")
        nc.scalar.copy(sxr[:], psx[:])
        sqr = ssp.tile([1, 512], f32r, tag="sqr")
        nc.scalar.copy(sqr[:], psq[:])
        psxb = psA.tile([P, 512], f32, tag="psA")
        nc.tensor.matmul(psxb[:], lhsT=ones_r32[:], rhs=sxr[:])
        psqb = psA.tile([P, 512], f32, tag="psA")
        nc.tensor.matmul(psqb[:], lhsT=ones_r32[:], rhs=sqr[:])
        mu_b = sp.tile([P, 512], f32, tag="t32")
        nc.scalar.activation(mu_b[:], psxb[:], AF.Copy, scale=1.0 / E)
        var_b = sp.tile([P, 512], f32, tag="t32")
        nc.scalar.activation(var_b[:], psqb[:], AF.Copy, scale=1.0 / E)
        tmp = sp.tile([P, 512], f32, tag="t32")
        nc.vector.tensor_mul(tmp[:], mu_b[:], mu_b[:])
        nc.vector.tensor_sub(var_b[:], var_b[:], tmp[:])
        nc.scalar.activation(var_b[:], var_b[:], AF.Sqrt, bias=eps128[:])
        rstd_b = sp.tile([P, 512], f32, tag="t32")
        nc.vector.reciprocal(rstd_b[:], var_b[:])
        ms_b = sp.tile([P, 512], f32, tag="t32")
        nc.vector.tensor_mul(ms_b[:], mu_b[:], rstd_b[:])
        for e in range(EC):
            t1 = sp.tile([P, 512], f32, tag="t32")
            nc.vector.tensor_mul(t1[:], mem[g][:, e, :], rstd_b[:])
            nc.vector.tensor_sub(t1[:], t1[:], ms_b[:])
            nc.vector.tensor_scalar(
                mn[:, e, :], t1[:], bcol(LNG, e), bcol(LNB, e),
                op0=Alu.mult, op1=Alu.add,
            )
        return mn

    # ---- phase 2: T recurrence steps.  The LN for step s+1 is emitted right
    # after group g's h' so it overlaps the other group's GRU matmuls; the
    # initial LNs overlap the KV phase tail.
    memn = [emit_ln(g) for g in range(NG)]
    for step in range(T):
        wq = load_w("wq")
        qt_g = []
        ot_g = []
        for g in range(NG):
            # Q projection for this group
            mn = memn[g]
            qt = bap.tile([P, EC, 512], f16, tag="ba")
            for of in range(EC):
                ps = psA.tile([P, 512], f32, tag="psA")
                for e in range(EC):
                    nc.tensor.matmul(
                        ps[:],
                        lhsT=wq[:, e, of * 128 : (of + 1) * 128],
                        rhs=mn[:, e, :],
                        start=(e == 0), stop=(e == EC - 1),
                    )
                nc.scalar.activation(qt[:, of, :], ps[:], AF.Identity, bias=bcol(BQ, of))
            qt_g.append(qt)

            # scores + softmax + A@V per batch of this group
            ot = otp.tile([P, EC, 512], f16, tag="ot")
            for bi in range(GB):
                b = g * GB + bi
                vt = vtp.tile([P, 4, E], f16, tag="vt")
                nc.sync.dma_start(vt[:], Vd[b][:])
                ps = psA.tile([P, L], f32, tag="psA")
                for e in range(EC):
                    nc.tensor.matmul(
                        ps[:],
                        lhsT=qt[:, e, bi * 128 : (bi + 1) * 128],
                        rhs=K[b][:, e, :],
                        start=(e == 0), stop=(e == EC - 1),
                    )
                aexp = sp.tile([P, L], f32, tag="aexp")
                rsum = ssp.tile([P, 1], f32, tag="rsum")
                nc.scalar.activation(aexp[:], ps[:], AF.Exp, accum_out=rsum[:])
                rinv = ssp.tile([P, 1], f32, tag="rinv")
                nc.vector.reciprocal(rinv[:], rsum[:])
                an = sp3.tile([P, L], f16, tag="s16")
                nc.vector.tensor_scalar_mul(an[:], aexp[:], rinv[:])
                att = sp.tile([P, 4, P], f16, tag="att")
                for kc in range(4):
                    pt = psT.tile([P, P], f16, tag="psT")
                    nc.tensor.transpose(pt[:], an[:, kc * 128 : (kc + 1) * 128], idy[:])
                    nc.vector.tensor_copy(att[:, kc, :], pt[:])
                for c in range(EC):
                    pb = psB.tile([P, P], f32, tag="psB")
                    for kc in range(4):
                        nc.tensor.matmul(
                            pb[:],
                            lhsT=vt[:, kc, c * 128 : (c + 1) * 128],
                            rhs=att[:, kc, :],
                            start=(kc == 0), stop=(kc == 3),
                        )
                    nc.scalar.copy(ot[:, c, bi * 128 : (bi + 1) * 128], pb[:])
            ot_g.append(ot)

        # GRU gates, r then z then n/h'. Wo is folded into the wih* weights on
        # the host (gi = O @ (Wih_g Wo).T + fused bias), so gates read ot_g.
        ut_g = ot_g
        wir = load_w("wihr")
        whr = load_w("whhr")
        rt_g = []
        for g in range(NG):
            rt = bap.tile([P, EC, 512], f16, tag="ba")
            for c in range(EC):
                ps = psA.tile([P, 512], f32, tag="psA")
                for e in range(EC):
                    nc.tensor.matmul(
                        ps[:], lhsT=wir[:, e, c * 128 : (c + 1) * 128],
                        rhs=ut_g[g][:, e, :], start=(e == 0), stop=False,
                    )
                for e in range(EC):
                    nc.tensor.matmul(
                        ps[:], lhsT=whr[:, e, c * 128 : (c + 1) * 128],
                        rhs=memn[g][:, e, :], start=False, stop=(e == EC - 1),
                    )
                nc.scalar.activation(rt[:, c, :], ps[:], AF.Sigmoid, bias=bcol(BR, c))
            rt_g.append(rt)
        wiz = load_w("wihz")
        whz = load_w("whhz")
        zt_g = []
        for g in range(NG):
            zg = bap.tile([P, EC, 512], f16, tag="ba")
            for c in range(EC):
                ps = psA.tile([P, 512], f32, tag="psA")
                for e in range(EC):
                    nc.tensor.matmul(
                        ps[:], lhsT=wiz[:, e, c * 128 : (c + 1) * 128],
                        rhs=ut_g[g][:, e, :], start=(e == 0), stop=False,
                    )
                for e in range(EC):
                    nc.tensor.matmul(
                        ps[:], lhsT=whz[:, e, c * 128 : (c + 1) * 128],
                        rhs=memn[g][:, e, :], start=False, stop=(e == EC - 1),
                    )
                nc.scalar.activation(zg[:, c, :], ps[:], AF.Sigmoid, bias=bcol(BZ, c))
            zt_g.append(zg)
        win = load_w("wihn")
        whn = load_w("whhn")
        new_memn = [None, None]
        for g in range(NG):
            for c in range(EC):
                psi = psA.tile([P, 512], f32, tag="psA")
                for e in range(EC):
                    nc.tensor.matmul(
                        psi[:], lhsT=win[:, e, c * 128 : (c + 1) * 128],
                        rhs=ut_g[g][:, e, :], start=(e == 0), stop=(e == EC - 1),
                    )
                psh = psA.tile([P, 512], f32, tag="psA")
                for e in range(EC):
                    nc.tensor.matmul(
                        psh[:], lhsT=whn[:, e, c * 128 : (c + 1) * 128],
                        rhs=memn[g][:, e, :], start=(e == 0), stop=(e == EC - 1),
                    )
                t1 = sp.tile([P, 512], f32, tag="t32")
                nc.vector.tensor_scalar_add(t1[:], psh[:], bcol(BHN, c))
                nc.vector.tensor_mul(t1[:], t1[:], rt_g[g][:, c, :])
                nc.vector.tensor_add(t1[:], t1[:], psi[:])
                ng = sp3.tile([P, 512], f16, tag="s16")
                nc.scalar.activation(ng[:], t1[:], AF.Tanh, bias=bcol(BIN, c))
                d = sp3.tile([P, 512], f16, tag="s16")
                nc.vector.tensor_sub(d[:], memn[g][:, c, :], ng[:])
                t2 = sp3.tile([P, 512], f16, tag="s16")
                nc.vector.tensor_mul(t2[:], zt_g[g][:, c, :], d[:])
                nc.vector.tensor_add(mem[g][:, c, :], ng[:], t2[:])
            if step < T - 1:
                new_memn[g] = emit_ln(g)
        memn = new_memn

    # ---- phase 3: read attention out = MHA(z, mem, mem)
    rwk = load_w("rwk")
    rwv = load_w("rwv")
    krt_g = []
    vrt_g = []
    for g in range(NG):
        krt = otp.tile([P, EC, 512], f16, tag="ot")
        for of in range(EC):
            ps = psA.tile([P, 512], f32, tag="psA")
            for e in range(EC):
                nc.tensor.matmul(
                    ps[:], lhsT=rwk[:, e, of * 128 : (of + 1) * 128],
                    rhs=mem[g][:, e, :], start=(e == 0), stop=(e == EC - 1),
                )
            nc.scalar.activation(krt[:, of, :], ps[:], AF.Identity, bias=bcol(RBK, of))
        krt_g.append(krt)
        vrt = mnp.tile([P, GB, E], f16, tag="mn")
        for bi in range(GB):
            for n0, nw in ((0, 512), (512, 256)):
                ps = psA.tile([P, nw], f32, tag="psA")
                for e in range(EC):
                    nc.tensor.matmul(
                        ps[:],
                        lhsT=mem[g][:, e, bi * 128 : (bi + 1) * 128],
                        rhs=rwv[:, e, n0 : n0 + nw],
                        start=(e == 0), stop=(e == EC - 1),
                    )
                nc.vector.tensor_add(
                    vrt[:, bi, n0 : n0 + nw], ps[:], brep16[:, E + n0 : E + n0 + nw]
                )
        vrt_g.append(vrt)

    rwq = load_w("rwq")
    rwo = load_w("rwo")
    for g in range(NG):
        for bi in range(GB):
            b = g * GB + bi
            zt = zp.tile([P, EC, L], f16, tag="zt")
            nc.sync.dma_start(zt[:], D["z"][b].rearrange("(c p) t -> p c t", p=P))
            qr = bap.tile([P, EC, L], f16, tag="ba")
            for of in range(EC):
                ps = psA.tile([P, L], f32, tag="psA")
                for e in range(EC):
                    nc.tensor.matmul(
                        ps[:], lhsT=rwq[:, e, of * 128 : (of + 1) * 128],
                        rhs=zt[:, e, :], start=(e == 0), stop=(e == EC - 1),
                    )
                nc.vector.tensor_scalar_add(qr[:, of, :], ps[:], bcol(RBQ, of))
            # scores^T [slot, tok]; softmax over slots = partition dim.
            # Normalization is deferred: O_r and the output projection run on
            # unnormalized exp scores; the per-token 1/colsum lands on the
            # token-major output via the ACT scale port.
            ps_s = psA.tile([P, L], f32, tag="psA")
            for of in range(EC):
                nc.tensor.matmul(
                    ps_s[:],
                    lhsT=krt_g[g][:, of, bi * 128 : (bi + 1) * 128],
                    rhs=qr[:, of, :],
                    start=(of == 0), stop=(of == EC - 1),
                )
            eS16 = sp3.tile([P, L], f16, tag="s16")
            nc.scalar.activation(eS16[:], ps_s[:], AF.Exp)
            rc4ps = psT.tile([P, 4], f32, tag="psT")
            for t4 in range(4):
                nc.tensor.matmul(
                    rc4ps[:, t4 : t4 + 1],
                    lhsT=eS16[:, t4 * 128 : (t4 + 1) * 128],
                    rhs=ones_c16[:],
                )
            rc4 = ssp.tile([P, 4], f32, tag="rc4")
            nc.vector.reciprocal(rc4[:], rc4ps[:])
            orr = bap.tile([P, EC, L], f16, tag="ba")
            for c in range(EC):
                pso = psB.tile([P, L], f32, tag="psB")
                nc.tensor.matmul(
                    pso[:], lhsT=vrt_g[g][:, bi, c * 128 : (c + 1) * 128], rhs=eS16[:]
                )
                nc.scalar.copy(orr[:, c, :], pso[:])
            for t4 in range(4):
                osb = op.tile([P, E], f32, tag="osb")
                for n0, nw in ((0, 512), (512, 256)):
                    ps = psA.tile([P, nw], f32, tag="psA")
                    for c in range(EC):
                        nc.tensor.matmul(
                            ps[:],
                            lhsT=orr[:, c, t4 * 128 : (t4 + 1) * 128],
                            rhs=rwo[:, c, n0 : n0 + nw],
                            start=(c == 0), stop=(c == EC - 1),
                        )
                    nc.scalar.activation(
                        osb[:, n0 : n0 + nw], ps[:], AF.Copy,
                        scale=rc4[:, t4 : t4 + 1],
                    )
                    nc.vector.tensor_add(
                        osb[:, n0 : n0 + nw], osb[:, n0 : n0 + nw],
                        brep16[:, 2 * E + n0 : 2 * E + n0 + nw],
                    )
                nc.sync.dma_start(D["out"][b, t4 * 128 : (t4 + 1) * 128, :], osb[:])


def _build():
    if "nc" in _CACHE:
        return _CACHE["nc"]
    nc = bacc.Bacc(
        "TRN2", target_bir_lowering=False, debug=False, enable_asserts=False
    )
    D = {}
    D["z"] = nc.dram_tensor("z", [NB, E, L], f16, kind="ExternalInput").ap()
    for name in (
        "wk", "wv", "wq",
        "wihr", "wihz", "wihn", "whhr", "whhz", "whhn",
        "rwq", "rwk", "rwv", "rwo",
    ):
        D[name] = nc.dram_tensor(name, [E, E], f16, kind="ExternalInput").ap()
    D["bias"] = nc.dram_tensor("bias", [P, 66], f32, kind="ExternalInput").ap()
    D["brep16"] = nc.dram_tensor("brep16", [P, 3 * E], f16, kind="ExternalInput").ap()
    D["slots"] = nc.dram_tensor("slots", [E, S], f16, kind="ExternalInput").ap()
    D["out"] = nc.dram_tensor("out", [NB, L, E], f32, kind="ExternalOutput").ap()
    with tile.TileContext(nc) as tc:
        with ExitStack() as ctx:
            _emit(nc, tc, ctx, D)
    nc.compile()
    _CACHE["nc"] = nc
    return nc


def _host_prep(inp):
    sq = 1.0 / np.sqrt(float(E))

    def t16(a):
        return np.ascontiguousarray(np.asarray(a).T).astype(np.float16)

    shared = {}
    shared["wk"] = t16(inp["w_wk"])
    shared["wv"] = t16(inp["w_wv"])
    shared["wq"] = t16(np.asarray(inp["w_wq"]) * sq)
    wo = np.asarray(inp["w_wo"], np.float64)
    wih = np.asarray(inp["gru_wih"], np.float64)
    whh = np.asarray(inp["gru_whh"])
    # Wo folded into the GRU input projections: gi_g = O @ (Wih_g Wo).T + b'
    shared["wihr"] = t16(wih[0:E] @ wo)
    shared["wihz"] = t16(wih[E : 2 * E] @ wo)
    shared["wihn"] = t16(wih[2 * E : 3 * E] @ wo)
    shared["whhr"] = t16(whh[0:E])
    shared["whhz"] = t16(whh[E : 2 * E])
    shared["whhn"] = t16(whh[2 * E : 3 * E])
    shared["rwq"] = t16(np.asarray(inp["r_wq"]) * sq)
    shared["rwk"] = t16(inp["r_wk"])
    shared["rwv"] = t16(inp["r_wv"])
    shared["rwo"] = t16(inp["r_wo"])

    def col6(v):
        return np.asarray(v, np.float32).reshape(EC, P).T

    bih = np.asarray(inp["gru_bih"], np.float64)
    bhh = np.asarray(inp["gru_bhh"], np.float64)
    bo = np.asarray(inp["w_bo"], np.float64)
    cols = [
        col6(inp["w_bk"]),
        col6(np.asarray(inp["w_bq"]) * sq),
        col6(bo),
        col6(wih[0:E] @ bo + bih[0:E] + bhh[0:E]),
        col6(wih[E : 2 * E] @ bo + bih[E : 2 * E] + bhh[E : 2 * E]),
        col6(wih[2 * E : 3 * E] @ bo + bih[2 * E : 3 * E]),
        col6(bhh[2 * E : 3 * E]),
        col6(np.asarray(inp["r_bq"]) * sq),
        col6(inp["r_bk"]),
        col6(inp["ln_g"]),
        col6(inp["ln_b"]),
    ]
    shared["bias"] = np.ascontiguousarray(np.concatenate(cols, axis=1), np.float32)
    bv = np.asarray(inp["w_bv"], np.float32)
    rbv = np.asarray(inp["r_bv"], np.float32)
    rbo = np.asarray(inp["r_bo"], np.float32)
    shared["brep16"] = np.ascontiguousarray(
        np.tile(np.concatenate([bv, rbv, rbo])[None, :], (P, 1)).astype(np.float16)
    )
    shared["slots"] = t16(np.asarray(inp["slots"])[0])

    z = np.asarray(inp["z"], np.float32)
    zt = np.ascontiguousarray(z.transpose(0, 2, 1)).astype(np.float16)
    in_maps = []
    for c in range(NCORE):
        m = dict(shared)
        m["z"] = np.ascontiguousarray(zt[c * NB : (c + 1) * NB])
        in_maps.append(m)
    return in_maps


def kernel(**inputs):
    nc = _build()
    in_maps = _host_prep(inputs)
    res = bass_utils.run_bass_kernel_spmd(nc, in_maps, core_ids=list(range(NCORE)))
    out = np.concatenate([res.results[c]["out"] for c in range(NCORE)], axis=0)
    return out.astype(np.float32)



# revision 3
# speedup vs baseline: 1.0944x; 1.0944x over previous
"""Trainium2 Bass kernel for nn_Memory_30571577213131 (scatter_memory).

Slot-memory module: T=3 recurrence steps of {LayerNorm -> write-MHA(mem, z, z)
-> GRUCell} followed by a read-MHA(z, mem, mem).

Sharding: pure data parallel - batch B=64 split as 8 batches per core across
8 NeuronCores; all parameters replicated.

v2: algebraic weight folding eliminates all z-sized projections.
  - Write attn: softmax over tokens is shift-invariant per slot, so
    scores = (LN(mem) @ (sq*Wq^T Wk) + sq*bq Wk) @ z^T  -- no K projection.
  - Softmax rows sum to 1, so A @ (z Wv^T + bv) = (A @ z) Wv^T + bv; Wv and
    Wo then fold into the GRU input weights: gi_g = (A@z) @ (Wih_g Wo Wv)^T.
    z token-major streams straight from DRAM (no V compute, no round-trip).
  - Read attn: scores^T = (mem @ Ar^T) @ z^T + (mem . cr) per-slot bias,
    Ar = (sq*Wq_r)^T Wk_r; the per-slot bias rides the Exp bias port.
    Wo_r Wv_r folds into the output projection: out = (A_r@mem) @ Wvo^T + b.
  - Output written fp16 (halves the store), upcast on host.

Layout: all activations feature-major ([128 feat-chunk, tokens/slots] with
6 chunks), matmuls at N=512; z resident feature-major for scores; mem
transposed on-chip (PE transposes) for the read-attention A@mem.
"""

import numpy as np
from contextlib import ExitStack

import concourse.bass as bass
import concourse.tile as tile
from concourse import bacc, mybir
from concourse import bass_utils
from concourse.masks import make_identity

f16 = mybir.dt.float16
f32 = mybir.dt.float32
f32r = mybir.dt.float32r
AF = mybir.ActivationFunctionType
Alu = mybir.AluOpType

P = 128
E = 768
EC = E // P          # 6 feature chunks
S = 128              # slots
T = 3                # recurrence steps
B = 64
L = 512
NCORE = 8
NB = B // NCORE      # 8 batches per core
GB = 4               # batches per group (4*128 slots = 512 free dim)
NG = NB // GB        # 2 groups
LN_EPS = 1e-5

# bias table column groups (each 6 wide) in the [128, 42] bias tile
BQ, BR, BZ, BIN, BHN, LNG, LNB = range(7)

_CACHE = {}


def _emit(nc, tc, ctx, D):
    cp = ctx.enter_context(tc.tile_pool(name="consts", bufs=1))
    wp = ctx.enter_context(tc.tile_pool(name="wts", bufs=3))
    zp = ctx.enter_context(tc.tile_pool(name="ztp", bufs=1))
    ztp = ctx.enter_context(tc.tile_pool(name="ztmp", bufs=2))
    mp = ctx.enter_context(tc.tile_pool(name="memp", bufs=1))
    mnp = ctx.enter_context(tc.tile_pool(name="memn", bufs=2))
    bap = ctx.enter_context(tc.tile_pool(name="bigact", bufs=4))
    otp = ctx.enter_context(tc.tile_pool(name="otp", bufs=2))
    mtp = ctx.enter_context(tc.tile_pool(name="mtp", bufs=1))
    anp = ctx.enter_context(tc.tile_pool(name="anp", bufs=4))
    esp = ctx.enter_context(tc.tile_pool(name="esp", bufs=1))
    sp = ctx.enter_context(tc.tile_pool(name="scratch", bufs=3))
    sp3 = ctx.enter_context(tc.tile_pool(name="scratch3", bufs=3))
    ssp = ctx.enter_context(tc.tile_pool(name="small", bufs=2))
    op = ctx.enter_context(tc.tile_pool(name="outp", bufs=2))
    psA = ctx.enter_context(tc.tile_pool(name="psA", bufs=4, space="PSUM"))
    psB = ctx.enter_context(tc.tile_pool(name="psB", bufs=2, space="PSUM"))
    psT = ctx.enter_context(tc.tile_pool(name="psT", bufs=2, space="PSUM"))

    # ---- constants
    idy = cp.tile([P, P], f16, tag="idy")
    make_identity(nc, idy[:])
    ones_c16 = cp.tile([P, 1], f16, tag="oc16")
    nc.vector.memset(ones_c16[:], 1.0)
    ones_r16 = cp.tile([1, P], f16, tag="or16")
    nc.vector.memset(ones_r16[:], 1.0)
    ones_c32 = cp.tile([P, 1], f32r, tag="oc32")
    nc.scalar.copy(ones_c32[:], ones_c16[:])
    ones_r32 = cp.tile([1, P], f32r, tag="or32")
    nc.scalar.copy(ones_r32[:], ones_r16[:])
    eps128 = cp.tile([P, 1], f32, tag="eps128")
    nc.vector.memset(eps128[:], LN_EPS)
    bias = cp.tile([P, 42], f32, tag="bias")
    nc.sync.dma_start(bias[:], D["bias"])
    brep16 = cp.tile([P, E], f16, tag="brep16")
    nc.sync.dma_start(brep16[:], D["brep16"])
    crt = cp.tile([P, EC], f16, tag="crt")
    nc.sync.dma_start(crt[:], D["crt"])

    def bcol(i, c):
        return bias[:, i * 6 + c : i * 6 + c + 1]

    def load_w(name):
        w = wp.tile([P, EC, E], f16, tag="w")
        nc.sync.dma_start(w[:], D[name].rearrange("(c p) f -> p c f", p=P))
        return w

    # ---- memory init from slots (broadcast to every batch)
    mem = []
    for g in range(NG):
        m = mp.tile([P, EC, 512], f16, tag=f"mem{g}")
        for bi in range(GB):
            nc.sync.dma_start(
                m[:, :, bi * 128 : (bi + 1) * 128],
                D["slots"].rearrange("(c p) s -> p c s", p=P),
            )
        mem.append(m)

    # ---- z feature-major, resident for all score matmuls
    zt = []
    for b in range(NB):
        z = zp.tile([P, EC, L], f16, tag=f"zt{b}")
        nc.sync.dma_start(z[:], D["z"][b].rearrange("(c p) t -> p c t", p=P))
        zt.append(z)

    # ---- LayerNorm emitter: partition-dim sums via ones-matmuls, then all
    # scalar math on [128,512] broadcast tiles (full DVE/ACT lane width).
    def emit_ln(g):
        mn = mnp.tile([P, EC, 512], f16, tag="mn")
        psx = psA.tile([1, 512], f32, tag="psA")
        for e in range(EC):
            nc.tensor.matmul(
                psx[:], lhsT=ones_c16[:], rhs=mem[g][:, e, :],
                start=(e == 0), stop=(e == EC - 1),
            )
        psq = psA.tile([1, 512], f32, tag="psA")
        for e in range(EC):
            sq = sp.tile([P, 512], f32r, tag="t32")
            nc.scalar.square(sq[:], mem[g][:, e, :])
            nc.tensor.matmul(
                psq[:], lhsT=ones_c32[:], rhs=sq[:],
                start=(e == 0), stop=(e == EC - 1),
            )
        sxr = ssp.tile([1, 512], f32r, tag="sxr")
        nc.scalar.copy(sxr[:], psx[:])
        sqr = ssp.tile([1, 512], f32r, tag="sqr")
        nc.scalar.copy(sqr[:], psq[:])
        psxb = psA.tile([P, 512], f32, tag="psA")
        nc.tensor.matmul(psxb[:], lhsT=ones_r32[:], rhs=sxr[:])
        psqb = psA.tile([P, 512], f32, tag="psA")
        nc.tensor.matmul(psqb[:], lhsT=ones_r32[:], rhs=sqr[:])
        mu_b = sp.tile([P, 512], f32, tag="t32")
        nc.scalar.activation(mu_b[:], psxb[:], AF.Copy, scale=1.0 / E)
        var_b = sp.tile([P, 512], f32, tag="t32")
        nc.scalar.activation(var_b[:], psqb[:], AF.Copy, scale=1.0 / E)
        tmp = sp.tile([P, 512], f32, tag="t32")
        nc.vector.tensor_mul(tmp[:], mu_b[:], mu_b[:])
        nc.vector.tensor_sub(var_b[:], var_b[:], tmp[:])
        nc.scalar.activation(var_b[:], var_b[:], AF.Sqrt, bias=eps128[:])
        rstd_b = sp.tile([P, 512], f32, tag="t32")
        nc.vector.reciprocal(rstd_b[:], var_b[:])
        ms_b = sp.tile([P, 512], f32, tag="t32")
        nc.vector.tensor_mul(ms_b[:], mu_b[:], rstd_b[:])
        for e in range(EC):
            t1 = sp.tile([P, 512], f32, tag="t32")
            nc.vector.tensor_mul(t1[:], mem[g][:, e, :], rstd_b[:])
            nc.vector.tensor_sub(t1[:], t1[:], ms_b[:])
            nc.vector.tensor_scalar(
                mn[:, e, :], t1[:], bcol(LNG, e), bcol(LNB, e),
                op0=Alu.mult, op1=Alu.add,
            )
        return mn

    # ---- recurrence: T steps of {qtilde -> scores vs z -> A@z -> GRU}
    memn = [emit_ln(g) for g in range(NG)]
    for step in range(T):
        wq = load_w("wq")
        ot_g = []
        for g in range(NG):
            # qtilde = LN(mem) @ Aw + cw  (write-attn Q/K folded)
            mn = memn[g]
            qt = bap.tile([P, EC, 512], f16, tag="ba")
            for of in range(EC):
                ps = psA.tile([P, 512], f32, tag="psA")
                for e in range(EC):
                    nc.tensor.matmul(
                        ps[:],
                        lhsT=wq[:, e, of * 128 : (of + 1) * 128],
                        rhs=mn[:, e, :],
                        start=(e == 0), stop=(e == EC - 1),
                    )
                nc.scalar.activation(qt[:, of, :], ps[:], AF.Identity, bias=bcol(BQ, of))

            # scores + softmax for all batches of the group first (keeps PE
            # streaming while ACT/DVE handle the softmax tails)
            an_l = []
            for bi in range(GB):
                b = g * GB + bi
                ps = psA.tile([P, L], f32, tag="psA")
                for e in range(EC):
                    nc.tensor.matmul(
                        ps[:],
                        lhsT=qt[:, e, bi * 128 : (bi + 1) * 128],
                        rhs=zt[b][:, e, :],
                        start=(e == 0), stop=(e == EC - 1),
                    )
                aexp = sp.tile([P, L], f32, tag="aexp")
                rsum = ssp.tile([P, 1], f32, tag="rsum")
                nc.scalar.activation(aexp[:], ps[:], AF.Exp, accum_out=rsum[:])
                rinv = ssp.tile([P, 1], f32, tag="rinv")
                nc.vector.reciprocal(rinv[:], rsum[:])
                an = anp.tile([P, L], f16, tag="an")
                nc.vector.tensor_scalar_mul(an[:], aexp[:], rinv[:])
                an_l.append(an)

            # ct = A @ z per batch (z token-major streamed from DRAM)
            ot = otp.tile([P, EC, 512], f16, tag="ot")
            for bi in range(GB):
                b = g * GB + bi
                zmt = ztp.tile([P, 4, E], f16, tag="zmt")
                nc.sync.dma_start(
                    zmt[:], D["ztm"][b].rearrange("(c p) f -> p c f", p=P)
                )
                att = sp3.tile([P, 4, P], f16, tag="att")
                for kc in range(4):
                    pt = psT.tile([P, P], f16, tag="psT")
                    nc.tensor.transpose(
                        pt[:], an_l[bi][:, kc * 128 : (kc + 1) * 128], idy[:]
                    )
                    nc.vector.tensor_copy(att[:, kc, :], pt[:])
                for c in range(EC):
                    pb = psB.tile([P, P], f32, tag="psB")
                    for kc in range(4):
                        nc.tensor.matmul(
                            pb[:],
                            lhsT=zmt[:, kc, c * 128 : (c + 1) * 128],
                            rhs=att[:, kc, :],
                            start=(kc == 0), stop=(kc == 3),
                        )
                    nc.scalar.copy(ot[:, c, bi * 128 : (bi + 1) * 128], pb[:])
            ot_g.append(ot)

        # GRU gates, r then z then n/h'. Wv and Wo are folded into the wih*
        # weights on the host (gi_g = (A@z) @ (Wih_g Wo Wv).T + fused bias).
        ut_g = ot_g
        wir = load_w("wihr")
        whr = load_w("whhr")
        rt_g = []
        for g in range(NG):
            rt = bap.tile([P, EC, 512], f16, tag="ba")
            for c in range(EC):
                ps = psA.tile([P, 512], f32, tag="psA")
                for e in range(EC):
                    nc.tensor.matmul(
                        ps[:], lhsT=wir[:, e, c * 128 : (c + 1) * 128],
                        rhs=ut_g[g][:, e, :], start=(e == 0), stop=False,
                    )
                for e in range(EC):
                    nc.tensor.matmul(
                        ps[:], lhsT=whr[:, e, c * 128 : (c + 1) * 128],
                        rhs=memn[g][:, e, :], start=False, stop=(e == EC - 1),
                    )
                nc.scalar.activation(rt[:, c, :], ps[:], AF.Sigmoid, bias=bcol(BR, c))
            rt_g.append(rt)
        wiz = load_w("wihz")
        whz = load_w("whhz")
        zt_g = []
        for g in range(NG):
            zg = bap.tile([P, EC, 512], f16, tag="ba")
            for c in range(EC):
                ps = psA.tile([P, 512], f32, tag="psA")
                for e in range(EC):
                    nc.tensor.matmul(
                        ps[:], lhsT=wiz[:, e, c * 128 : (c + 1) * 128],
                        rhs=ut_g[g][:, e, :], start=(e == 0), stop=False,
                    )
                for e in range(EC):
                    nc.tensor.matmul(
                        ps[:], lhsT=whz[:, e, c * 128 : (c + 1) * 128],
                        rhs=memn[g][:, e, :], start=False, stop=(e == EC - 1),
                    )
                nc.scalar.activation(zg[:, c, :], ps[:], AF.Sigmoid, bias=bcol(BZ, c))
            zt_g.append(zg)
        win = load_w("wihn")
        whn = load_w("whhn")
        new_memn = [None, None]
        for g in range(NG):
            for c in range(EC):
                psi = psA.tile([P, 512], f32, tag="psA")
                for e in range(EC):
                    nc.tensor.matmul(
                        psi[:], lhsT=win[:, e, c * 128 : (c + 1) * 128],
                        rhs=ut_g[g][:, e, :], start=(e == 0), stop=(e == EC - 1),
                    )
                psh = psA.tile([P, 512], f32, tag="psA")
                for e in range(EC):
                    nc.tensor.matmul(
                        psh[:], lhsT=whn[:, e, c * 128 : (c + 1) * 128],
                        rhs=memn[g][:, e, :], start=(e == 0), stop=(e == EC - 1),
                    )
                t1 = sp.tile([P, 512], f32, tag="t32")
                nc.vector.tensor_scalar_add(t1[:], psh[:], bcol(BHN, c))
                nc.vector.tensor_mul(t1[:], t1[:], rt_g[g][:, c, :])
                nc.vector.tensor_add(t1[:], t1[:], psi[:])
                ng = sp3.tile([P, 512], f16, tag="s16")
                nc.scalar.activation(ng[:], t1[:], AF.Tanh, bias=bcol(BIN, c))
                d = sp3.tile([P, 512], f16, tag="s16")
                nc.vector.tensor_sub(d[:], memn[g][:, c, :], ng[:])
                t2 = sp3.tile([P, 512], f16, tag="s16")
                nc.vector.tensor_mul(t2[:], zt_g[g][:, c, :], d[:])
                nc.vector.tensor_add(mem[g][:, c, :], ng[:], t2[:])
            if step < T - 1:
                new_memn[g] = emit_ln(g)
        memn = new_memn

    # ---- phase 3: read attention out = MHA(z, mem, mem), Q/K and V/O folded
    rwk = load_w("rwk")   # Ar^T: mt = mem @ Ar^T
    rwo = load_w("rwo")   # Wvo^T: out = O_r @ Wvo^T
    mt_g = []
    dcol_g = []
    mT_g = []
    for g in range(NG):
        mt = otp.tile([P, EC, 512], f16, tag="ot")
        for of in range(EC):
            ps = psA.tile([P, 512], f32, tag="psA")
            for e in range(EC):
                nc.tensor.matmul(
                    ps[:], lhsT=rwk[:, e, of * 128 : (of + 1) * 128],
                    rhs=mem[g][:, e, :], start=(e == 0), stop=(e == EC - 1),
                )
            nc.scalar.copy(mt[:, of, :], ps[:])
        mt_g.append(mt)

        # per-slot read-score bias d_s = cr . mem_s -> [128,1] columns
        psd = psA.tile([1, 512], f32, tag="psA")
        for e in range(EC):
            nc.tensor.matmul(
                psd[:], lhsT=crt[:, e : e + 1], rhs=mem[g][:, e, :],
                start=(e == 0), stop=(e == EC - 1),
            )
        sd = sp3.tile([1, 512], f16, tag="sd")
        nc.scalar.copy(sd[:], psd[:])
        psdT = psT.tile([P, 4], f32, tag="psT")
        for bi in range(GB):
            nc.tensor.matmul(
                psdT[:, bi : bi + 1],
                lhsT=sd[0:1, bi * 128 : (bi + 1) * 128],
                rhs=ones_r16[0:1, 0:1],
            )
        dcol = ssp.tile([P, 4], f32, tag=f"dcol{g}")
        nc.scalar.copy(dcol[:], psdT[:])
        dcol_g.append(dcol)

        # mem transposed to slot-major for the A_r @ mem matmuls
        mT = mtp.tile([P, 4, E], f16, tag=f"mT{g}")
        for bi in range(GB):
            for e in range(EC):
                pt = psT.tile([P, P], f16, tag="psT")
                nc.tensor.transpose(pt[:], mem[g][:, e, bi * 128 : (bi + 1) * 128], idy[:])
                if e % 2 == 0:
                    nc.vector.tensor_copy(mT[:, bi, e * 128 : (e + 1) * 128], pt[:])
                else:
                    nc.scalar.copy(mT[:, bi, e * 128 : (e + 1) * 128], pt[:])
        mT_g.append(mT)

    # scores for all batches first, then the per-batch value/output chains
    eS_l = []
    for b in range(NB):
        g, bi = b // GB, b % GB
        ps_s = psA.tile([P, L], f32, tag="psA")
        for of in range(EC):
            nc.tensor.matmul(
                ps_s[:],
                lhsT=mt_g[g][:, of, bi * 128 : (bi + 1) * 128],
                rhs=zt[b][:, of, :],
                start=(of == 0), stop=(of == EC - 1),
            )
        eS = esp.tile([P, L], f16, tag=f"es{b}")
        nc.scalar.activation(eS[:], ps_s[:], AF.Exp, bias=dcol_g[g][:, bi : bi + 1])
        eS_l.append(eS)

    for b in range(NB):
        g, bi = b // GB, b % GB
        eS = eS_l[b]
        # per-token 1/colsum via ones-matmul + tiny transposes
        psc = psA.tile([1, L], f32, tag="psA")
        nc.tensor.matmul(psc[:], lhsT=ones_c16[:], rhs=eS[:])
        sc = sp3.tile([1, L], f16, tag="sd")
        nc.scalar.copy(sc[:], psc[:])
        psrT = psT.tile([P, 4], f32, tag="psT")
        for t4 in range(4):
            nc.tensor.matmul(
                psrT[:, t4 : t4 + 1],
                lhsT=sc[0:1, t4 * 128 : (t4 + 1) * 128],
                rhs=ones_r16[0:1, 0:1],
            )
        rc4 = ssp.tile([P, 4], f32, tag="rc4")
        nc.vector.reciprocal(rc4[:], psrT[:])
        # O_r^T = mem^T @ A^T on unnormalized exp scores
        orr = bap.tile([P, EC, L], f16, tag="ba")
        for c in range(EC):
            pso = psB.tile([P, L], f32, tag="psB")
            nc.tensor.matmul(
                pso[:], lhsT=mT_g[g][:, bi, c * 128 : (c + 1) * 128], rhs=eS[:]
            )
            nc.vector.tensor_copy(orr[:, c, :], pso[:])
        # out = O_r @ Wvo^T * (1/colsum) + b_out, token-major, fp16 store
        for t4 in range(4):
            osb = op.tile([P, E], f16, tag="osb")
            for n0, nw in ((0, 512), (512, 256)):
                ps = psA.tile([P, nw], f32, tag="psA")
                for c in range(EC):
                    nc.tensor.matmul(
                        ps[:],
                        lhsT=orr[:, c, t4 * 128 : (t4 + 1) * 128],
                        rhs=rwo[:, c, n0 : n0 + nw],
                        start=(c == 0), stop=(c == EC - 1),
                    )
                nc.scalar.activation(
                    osb[:, n0 : n0 + nw], ps[:], AF.Copy,
                    scale=rc4[:, t4 : t4 + 1],
                )
                nc.vector.tensor_add(
                    osb[:, n0 : n0 + nw], osb[:, n0 : n0 + nw],
                    brep16[:, n0 : n0 + nw],
                )
            nc.sync.dma_start(D["out"][b, t4 * 128 : (t4 + 1) * 128, :], osb[:])


def _build():
    if "nc" in _CACHE:
        return _CACHE["nc"]
    nc = bacc.Bacc(
        "TRN2", target_bir_lowering=False, debug=False, enable_asserts=False
    )
    D = {}
    D["z"] = nc.dram_tensor("z", [NB, E, L], f16, kind="ExternalInput").ap()
    D["ztm"] = nc.dram_tensor("ztm", [NB, L, E], f16, kind="ExternalInput").ap()
    for name in (
        "wq",
        "wihr", "wihz", "wihn", "whhr", "whhz", "whhn",
        "rwk", "rwo",
    ):
        D[name] = nc.dram_tensor(name, [E, E], f16, kind="ExternalInput").ap()
    D["bias"] = nc.dram_tensor("bias", [P, 42], f32, kind="ExternalInput").ap()
    D["brep16"] = nc.dram_tensor("brep16", [P, E], f16, kind="ExternalInput").ap()
    D["crt"] = nc.dram_tensor("crt", [P, EC], f16, kind="ExternalInput").ap()
    D["slots"] = nc.dram_tensor("slots", [E, S], f16, kind="ExternalInput").ap()
    D["out"] = nc.dram_tensor("out", [NB, L, E], f16, kind="ExternalOutput").ap()
    with tile.TileContext(nc) as tc:
        with ExitStack() as ctx:
            _emit(nc, tc, ctx, D)
    nc.compile()
    _CACHE["nc"] = nc
    return nc


def _host_prep(inp):
    sq = 1.0 / np.sqrt(float(E))
    f64 = np.float64

    def t16(a):
        return np.ascontiguousarray(np.asarray(a).T).astype(np.float16)

    def col6(v):
        return np.asarray(v, np.float32).reshape(EC, P).T

    wq_w = np.asarray(inp["w_wq"], f64)
    wk_w = np.asarray(inp["w_wk"], f64)
    wv_w = np.asarray(inp["w_wv"], f64)
    wo_w = np.asarray(inp["w_wo"], f64)
    bq_w = np.asarray(inp["w_bq"], f64)
    bv_w = np.asarray(inp["w_bv"], f64)
    bo_w = np.asarray(inp["w_bo"], f64)
    wih = np.asarray(inp["gru_wih"], f64)
    whh = np.asarray(inp["gru_whh"], f64)
    bih = np.asarray(inp["gru_bih"], f64)
    bhh = np.asarray(inp["gru_bhh"], f64)
    wq_r = np.asarray(inp["r_wq"], f64)
    wk_r = np.asarray(inp["r_wk"], f64)
    wv_r = np.asarray(inp["r_wv"], f64)
    wo_r = np.asarray(inp["r_wo"], f64)
    bq_r = np.asarray(inp["r_bq"], f64)
    bv_r = np.asarray(inp["r_bv"], f64)
    bo_r = np.asarray(inp["r_bo"], f64)

    shared = {}
    # write attention Q/K folded: qtilde = mln @ Aw + cw
    Aw = (sq * wq_w).T @ wk_w                    # [mln-feat, z-feat]
    cw = (sq * bq_w) @ wk_w                      # [z-feat]
    shared["wq"] = np.ascontiguousarray(Aw).astype(np.float16)
    # GRU input side with Wo Wv folded: gi_g = (A@z) @ (Wih_g Wo Wv)^T
    wov = wo_w @ wv_w                            # [E, z-feat]
    bov = wo_w @ bv_w + bo_w                     # [E]
    shared["wihr"] = t16(wih[0:E] @ wov)
    shared["wihz"] = t16(wih[E : 2 * E] @ wov)
    shared["wihn"] = t16(wih[2 * E : 3 * E] @ wov)
    shared["whhr"] = t16(whh[0:E])
    shared["whhz"] = t16(whh[E : 2 * E])
    shared["whhn"] = t16(whh[2 * E : 3 * E])
    # read attention: mt = mem @ Ar^T, d = mem . cr, out = O_r @ Wvo^T + bout
    Ar = (sq * wq_r).T @ wk_r                    # [z-feat, mem-feat]
    cr = (sq * bq_r) @ wk_r                      # [mem-feat]
    shared["rwk"] = np.ascontiguousarray(wk_r.T @ (sq * wq_r)).astype(np.float16)
    wvo = wo_r @ wv_r                            # [out-feat, mem-feat]
    bout = wo_r @ bv_r + bo_r                    # [out-feat]
    shared["rwo"] = t16(wvo)
    shared["crt"] = np.ascontiguousarray(col6(cr)).astype(np.float16)

    cols = [
        col6(cw),
        col6(wih[0:E] @ bov + bih[0:E] + bhh[0:E]),
        col6(wih[E : 2 * E] @ bov + bih[E : 2 * E] + bhh[E : 2 * E]),
        col6(wih[2 * E : 3 * E] @ bov + bih[2 * E : 3 * E]),
        col6(bhh[2 * E : 3 * E]),
        col6(inp["ln_g"]),
        col6(inp["ln_b"]),
    ]
    shared["bias"] = np.ascontiguousarray(np.concatenate(cols, axis=1), np.float32)
    shared["brep16"] = np.ascontiguousarray(
        np.tile(bout[None, :], (P, 1)).astype(np.float16)
    )
    shared["slots"] = t16(np.asarray(inp["slots"])[0])

    z = np.asarray(inp["z"], np.float32)
    zt = np.ascontiguousarray(z.transpose(0, 2, 1)).astype(np.float16)
    in_maps = []
    for c in range(NCORE):
        m = dict(shared)
        m["z"] = np.ascontiguousarray(zt[c * NB : (c + 1) * NB])
        m["ztm"] = np.ascontiguousarray(z[c * NB : (c + 1) * NB].astype(np.float16))
        in_maps.append(m)
    return in_maps


def kernel(**inputs):
    nc = _build()
    in_maps = _host_prep(inputs)
    res = bass_utils.run_bass_kernel_spmd(nc, in_maps, core_ids=list(range(NCORE)))
    out = np.concatenate([res.results[c]["out"] for c in range(NCORE)], axis=0)
    return out.astype(np.float32)


# revision 19
# speedup vs baseline: 1.3967x; 1.2763x over previous
"""Trainium2 Bass kernel for nn_Memory_30571577213131 (scatter_memory).

Slot-memory module: T=3 recurrence steps of {LayerNorm -> write-MHA(mem, z, z)
-> GRUCell} followed by a read-MHA(z, mem, mem).

Sharding: pure data parallel - batch B=64 split as 8 batches per core across
8 NeuronCores; all parameters replicated.

v3: algebraic weight folding eliminates all z-sized projections (write-attn
Q/K folded into one matrix vs LN(mem); A@V = (A@z) Wv^T via softmax row-sum=1
with Wv,Wo folded into GRU input weights; read-attn Q/K folded, Wv_r,Wo_r
folded into the output projection).  z streams token-major for A@z; z
feature-major is resident for all score matmuls.  LayerNorm stats run on
[1,512] rows with the gamma/beta application fused into 2 DVE ops per chunk
via a K=2 broadcast matmul.  GRU elementwise tail uses scalar_tensor_tensor
and alternates DVE/GpSimd.  A@z runs slot-major at N=512/256 with the
softmax normalization folded into the PSUM->SBUF copy, then PE transposes
back to feature-major.  wq is resident; GRU weights prefetch early.
"""

import numpy as np
import ml_dtypes
from contextlib import ExitStack

import concourse.bass as bass
import concourse.tile as tile
from concourse import bacc, mybir
from concourse import bass_utils
from concourse.masks import make_identity

f16 = mybir.dt.float16
f32 = mybir.dt.float32
f32r = mybir.dt.float32r
f8 = mybir.dt.float8e4
f8e5 = mybir.dt.float8e5
AF = mybir.ActivationFunctionType
Alu = mybir.AluOpType

P = 128
E = 768
EC = E // P          # 6 feature chunks
S = 128              # slots
T = 3                # recurrence steps
B = 64
L = 512
NCORE = 8
NB = B // NCORE      # 8 batches per core
GB = 4               # batches per group (4*128 slots = 512 free dim)
NG = NB // GB        # 2 groups
LN_EPS = 1e-5

# bias table column groups (each 6 wide) in the [128, 42] bias tile
BQ, BR, BZ, BIN, BHN, LNG, LNB = range(7)

_CACHE = {}


def _emit(nc, tc, ctx, D):
    cp = ctx.enter_context(tc.tile_pool(name="consts", bufs=1))
    wres = ctx.enter_context(tc.tile_pool(name="wres", bufs=1))
    wp = ctx.enter_context(tc.tile_pool(name="wts", bufs=3))
    zp = ctx.enter_context(tc.tile_pool(name="ztp", bufs=1))
    ztp = ctx.enter_context(tc.tile_pool(name="ztmp", bufs=2))
    mp = ctx.enter_context(tc.tile_pool(name="memp", bufs=1))
    mnp = ctx.enter_context(tc.tile_pool(name="memn", bufs=2))
    bap = ctx.enter_context(tc.tile_pool(name="bigact", bufs=4))
    otp = ctx.enter_context(tc.tile_pool(name="otp", bufs=2))
    mtp = ctx.enter_context(tc.tile_pool(name="mtp", bufs=1))
    ewp = ctx.enter_context(tc.tile_pool(name="esw", bufs=3))
    cmp_ = ctx.enter_context(tc.tile_pool(name="ctm", bufs=1))
    sp = ctx.enter_context(tc.tile_pool(name="scratch", bufs=2))
    sp3 = ctx.enter_context(tc.tile_pool(name="scratch3", bufs=3))
    atp = ctx.enter_context(tc.tile_pool(name="attp", bufs=1))
    ssp = ctx.enter_context(tc.tile_pool(name="small", bufs=1))
    tp = ctx.enter_context(tc.tile_pool(name="tiny", bufs=2))
    op = ctx.enter_context(tc.tile_pool(name="outp", bufs=2))
    psA = ctx.enter_context(tc.tile_pool(name="psA", bufs=4, space="PSUM"))
    psB = ctx.enter_context(tc.tile_pool(name="psB", bufs=2, space="PSUM"))
    psT = ctx.enter_context(tc.tile_pool(name="psT", bufs=2, space="PSUM"))

    # ---- constants
    idy = cp.tile([P, P], f16, tag="idy")
    make_identity(nc, idy[:])
    ones_c16 = cp.tile([P, 1], f16, tag="oc16")
    nc.vector.memset(ones_c16[:], 1.0)
    ones_r16 = cp.tile([1, P], f16, tag="or16")
    nc.vector.memset(ones_r16[:], 1.0)
    eps128 = cp.tile([P, 1], f32, tag="eps128")
    nc.vector.memset(eps128[:], LN_EPS)
    bias = cp.tile([P, 42], f32, tag="bias")
    nc.sync.dma_start(bias[:], D["bias"])
    brep16 = cp.tile([P, E], f16, tag="brep16")
    nc.sync.dma_start(brep16[:], D["brep16"])
    crt = cp.tile([P, EC], f16, tag="crt")
    nc.sync.dma_start(crt[:], D["crt"])
    bgl = cp.tile([2, E], f16, tag="bgl")
    nc.sync.dma_start(bgl[:], D["bgl"])
    # vrow row1 = ones (written once via DMA; row0 = mu*rstd per LN call)
    vrow = cp.tile([2, 512], f16, tag="vrow")
    nc.sync.dma_start(vrow[1:2, :], D["onesrow"])

    def bcol(i, c):
        return bias[:, i * 6 + c : i * 6 + c + 1]

    def load_w(name, pool=None, tag="w"):
        w = (pool or wp).tile([P, EC, E], f16, tag=tag)
        nc.sync.dma_start(w[:], D[name].rearrange("(c p) f -> p c f", p=P))
        return w

    def load_w8(name):
        w = wp.tile([P, 3, 2, E], f8e5, tag="w8")
        nc.sync.dma_start(w[:], D[name])
        return w

    # wq (folded write-attn QK matrix) is reused every step: resident
    wq = load_w("wq", pool=wres, tag="wq")

    # ---- memory init from slots (broadcast to every batch)
    mem = []
    for g in range(NG):
        m = mp.tile([P, EC, 512], f16, tag=f"mem{g}")
        for bi in range(GB):
            nc.sync.dma_start(
                m[:, :, bi * 128 : (bi + 1) * 128],
                D["slots"].rearrange("(c p) s -> p c s", p=P),
            )
        mem.append(m)

    # ---- z feature-major, resident for all score matmuls
    zt = []
    for b in range(NB):
        z = zp.tile([P, EC, L], f16, tag=f"zt{b}")
        nc.sync.dma_start(z[:], D["z"][b].rearrange("(c p) t -> p c t", p=P))
        zt.append(z)

    # ---- LayerNorm: row stats + K=2 broadcast matmul (g*mu*rstd - b), then
    # two fused DVE ops per chunk.
    def emit_ln(g):
        mn = mnp.tile([P, EC, 512], f16, tag="mn")
        psx = psA.tile([1, 512], f32, tag="psA")
        for e in range(EC):
            nc.tensor.matmul(
                psx[:], lhsT=ones_c16[:], rhs=mem[g][:, e, :],
                start=(e == 0), stop=(e == EC - 1),
            )
        psq = psA.tile([1, 512], f32, tag="psA")
        for e in range(EC):
            sq = sp.tile([P, 512], f16, tag="t32")
            nc.vector.tensor_mul(sq[:], mem[g][:, e, :], mem[g][:, e, :])
            nc.tensor.matmul(
                psq[:], lhsT=ones_c16[:], rhs=sq[:],
                start=(e == 0), stop=(e == EC - 1),
            )
        mu = ssp.tile([1, 512], f16, tag="mu")
        nc.scalar.activation(mu[:], psx[:], AF.Copy, scale=1.0 / E)
        r1 = ssp.tile([1, 512], f16, tag="r1")
        r2 = ssp.tile([1, 512], f16, tag="r2")
        with nc.allow_low_precision(reason="LN row stats in f16 feed f16 math"):
            nc.vector.tensor_mul(r1[:], mu[:], mu[:])                       # mu^2
            nc.vector.scalar_tensor_tensor(
                r2[:], psq[:], 1.0 / E, r1[:], op0=Alu.mult, op1=Alu.subtract
            )                                                               # var
            nc.scalar.activation(r2[:], r2[:], AF.Sqrt, bias=eps128[0:1, :])
            nc.vector.reciprocal(r1[:], r2[:])                              # rstd
            nc.vector.tensor_mul(vrow[0:1, :], mu[:], r1[:])                # mu*rstd
        psr = psA.tile([P, 512], f32, tag="psA")
        nc.tensor.matmul(psr[:], lhsT=ones_r16[:], rhs=r1[:])
        rstd_b = sp.tile([P, 512], f16, tag="rb16")
        nc.scalar.copy(rstd_b[:], psr[:])
        for e in range(EC):
            psv = psA.tile([P, 512], f32, tag="psA")
            nc.tensor.matmul(psv[:], lhsT=bgl[:, e * 128 : (e + 1) * 128], rhs=vrow[:])
            u = sp3.tile([P, 512], f16, tag="s16")
            nc.vector.scalar_tensor_tensor(
                u[:], mem[g][:, e, :], bcol(LNG, e), rstd_b[:],
                op0=Alu.mult, op1=Alu.mult,
            )
            nc.vector.scalar_tensor_tensor(
                mn[:, e, :], psv[:], -1.0, u[:], op0=Alu.mult, op1=Alu.add,
            )
        return mn

    # ---- recurrence
    memn = [emit_ln(0), emit_ln(1)]
    for step in range(T):
        wir = load_w8("wihr8")
        whr = load_w("whhr")
        ot_g = []
        for g in range(NG):
            if memn[g] is None:
                memn[g] = emit_ln(g)   # pipelined: overlaps prev group's work
            mn = memn[g]
            # qtilde = LN(mem) @ Aw + cw  (write-attn Q/K folded)
            qt = bap.tile([P, EC, 512], f16, tag="ba")
            for of in range(EC):
                ps = psA.tile([P, 512], f32, tag="psA")
                for e in range(EC):
                    nc.tensor.matmul(
                        ps[:],
                        lhsT=wq[:, e, of * 128 : (of + 1) * 128],
                        rhs=mn[:, e, :],
                        start=(e == 0), stop=(e == EC - 1),
                    )
                nc.scalar.activation(qt[:, of, :], ps[:], AF.Identity, bias=bcol(BQ, of))

            # scores -> unnormalized exp + row sums, whole group first
            eS_l = []
            rinv_l = []
            for bi in range(GB):
                b = g * GB + bi
                ps = psA.tile([P, L], f32, tag="psA")
                for e in range(EC):
                    nc.tensor.matmul(
                        ps[:],
                        lhsT=qt[:, e, bi * 128 : (bi + 1) * 128],
                        rhs=zt[b][:, e, :],
                        start=(e == 0), stop=(e == EC - 1),
                    )
                eS = ewp.tile([P, L], f16, tag="esw")
                rsum = tp.tile([P, 1], f32, tag="rsum")
                nc.scalar.activation(eS[:], ps[:], AF.Exp, accum_out=rsum[:])
                rinv = tp.tile([P, 1], f32, tag="rinv")
                nc.vector.reciprocal(rinv[:], rsum[:])
                rinv8 = tp.tile([P, 1], f32, tag="rinv8")
                nc.vector.tensor_scalar_mul(rinv8[:], rinv[:], 8.0)
                eS_l.append(eS)
                rinv_l.append(rinv8)

            # ct = A @ z slot-major (N=512/256), normalization folded into the
            # PSUM->SBUF copy, then PE transposes back to feature-major
            ot = otp.tile([P, EC, 512], f8, tag="ot")
            for bi in range(GB):
                b = g * GB + bi
                zmt = ztp.tile([P, 4, E], f16, tag="zmt")
                nc.sync.dma_start(
                    zmt[:], D["ztm"][b].rearrange("(c p) f -> p c f", p=P)
                )
                att = atp.tile([P, 4, P], f16, tag="att")
                for kc in range(4):
                    pt = psT.tile([P, P], f16, tag="psT")
                    nc.tensor.transpose(
                        pt[:], eS_l[bi][:, kc * 128 : (kc + 1) * 128], idy[:]
                    )
                    nc.vector.tensor_copy(att[:, kc, :], pt[:])
                ps1 = psB.tile([P, 512], f32, tag="psB")
                ps2 = psB.tile([P, 256], f32, tag="psB")
                for kc in range(4):
                    nc.tensor.matmul(
                        ps1[:], lhsT=att[:, kc, :], rhs=zmt[:, kc, 0:512],
                        start=(kc == 0), stop=(kc == 3),
                    )
                    nc.tensor.matmul(
                        ps2[:], lhsT=att[:, kc, :], rhs=zmt[:, kc, 512:768],
                        start=(kc == 0), stop=(kc == 3),
                    )
                ctm = cmp_.tile([P, E], f16, tag="ctm")
                nc.scalar.activation(
                    ctm[:, 0:512], ps1[:], AF.Copy, scale=rinv_l[bi][:]
                )
                nc.scalar.activation(
                    ctm[:, 512:768], ps2[:], AF.Copy, scale=rinv_l[bi][:]
                )
                for c in range(EC):
                    pt = psT.tile([P, P], f16, tag="psT")
                    nc.tensor.transpose(pt[:], ctm[:, c * 128 : (c + 1) * 128], idy[:])
                    if c % 2 == 0:
                        nc.vector.tensor_copy(ot[:, c, bi * 128 : (bi + 1) * 128], pt[:])
                    else:
                        nc.scalar.copy(ot[:, c, bi * 128 : (bi + 1) * 128], pt[:])
            ot_g.append(ot)

        # GRU gates, r then z then n/h'.
        ut_g = ot_g
        wiz = load_w8("wihz8")
        whz = load_w("whhz")
        rt_g = []
        for g in range(NG):
            rt = bap.tile([P, EC, 512], f16, tag="ba")
            for c in range(EC):
                ps = psA.tile([P, 512], f32, tag="psA")
                for j in range(3):
                    nc.tensor.matmul(
                        ps[:], lhsT=wir[:, j, :, c * 128 : (c + 1) * 128],
                        rhs=ut_g[g][:, 2 * j : 2 * j + 2, :],
                        start=(j == 0), stop=False,
                        perf_mode=mybir.MatmulPerfMode.DoubleRow,
                    )
                for e in range(EC):
                    nc.tensor.matmul(
                        ps[:], lhsT=whr[:, e, c * 128 : (c + 1) * 128],
                        rhs=memn[g][:, e, :], start=False, stop=(e == EC - 1),
                    )
                nc.scalar.activation(rt[:, c, :], ps[:], AF.Sigmoid, bias=bcol(BR, c))
            rt_g.append(rt)
        win = load_w8("wihn8")
        whn = load_w("whhn")
        zt_g = []
        for g in range(NG):
            zg = bap.tile([P, EC, 512], f16, tag="ba")
            for c in range(EC):
                ps = psA.tile([P, 512], f32, tag="psA")
                for j in range(3):
                    nc.tensor.matmul(
                        ps[:], lhsT=wiz[:, j, :, c * 128 : (c + 1) * 128],
                        rhs=ut_g[g][:, 2 * j : 2 * j + 2, :],
                        start=(j == 0), stop=False,
                        perf_mode=mybir.MatmulPerfMode.DoubleRow,
                    )
                for e in range(EC):
                    nc.tensor.matmul(
                        ps[:], lhsT=whz[:, e, c * 128 : (c + 1) * 128],
                        rhs=memn[g][:, e, :], start=False, stop=(e == EC - 1),
                    )
                nc.scalar.activation(zg[:, c, :], ps[:], AF.Sigmoid, bias=bcol(BZ, c))
            zt_g.append(zg)
        if step == T - 1:
            rwk = load_w("rwk")   # prefetch for phase 3
            rwo = load_w("rwo")
        for g in range(NG):
            for c in range(EC):
                psi = psA.tile([P, 512], f32, tag="psA")
                for j in range(3):
                    nc.tensor.matmul(
                        psi[:], lhsT=win[:, j, :, c * 128 : (c + 1) * 128],
                        rhs=ut_g[g][:, 2 * j : 2 * j + 2, :],
                        start=(j == 0), stop=(j == 2),
                        perf_mode=mybir.MatmulPerfMode.DoubleRow,
                    )
                psh = psA.tile([P, 512], f32, tag="psA")
                for e in range(EC):
                    nc.tensor.matmul(
                        psh[:], lhsT=whn[:, e, c * 128 : (c + 1) * 128],
                        rhs=memn[g][:, e, :], start=(e == 0), stop=(e == EC - 1),
                    )
                t1 = sp.tile([P, 512], f32, tag="tf")
                nc.vector.scalar_tensor_tensor(
                    t1[:], psh[:], bcol(BHN, c), rt_g[g][:, c, :],
                    op0=Alu.add, op1=Alu.mult,
                )
                t2 = sp.tile([P, 512], f32, tag="tf")
                nc.vector.tensor_add(t2[:], t1[:], psi[:])
                ng = sp3.tile([P, 512], f16, tag="s16")
                nc.scalar.activation(ng[:], t2[:], AF.Tanh, bias=bcol(BIN, c))
                eng = nc.gpsimd if (c % 2 == 0) else nc.vector
                d = sp3.tile([P, 512], f16, tag="s16")
                eng.tensor_sub(d[:], memn[g][:, c, :], ng[:])
                t3 = sp3.tile([P, 512], f16, tag="s16")
                eng.tensor_mul(t3[:], zt_g[g][:, c, :], d[:])
                eng.tensor_add(mem[g][:, c, :], ng[:], t3[:])
            if step < T - 1:
                if g == 0:
                    memn[0] = emit_ln(0)
                else:
                    memn[1] = None     # emitted inside next step's group loop

    # ---- phase 3: read attention out = MHA(z, mem, mem), Q/K and V/O folded
    mt_g = []
    dcol_g = []
    mT_g = []
    for g in range(NG):
        mt = mnp.tile([P, EC, 512], f16, tag="mn")
        for of in range(EC):
            ps = psA.tile([P, 512], f32, tag="psA")
            for e in range(EC):
                nc.tensor.matmul(
                    ps[:], lhsT=rwk[:, e, of * 128 : (of + 1) * 128],
                    rhs=mem[g][:, e, :], start=(e == 0), stop=(e == EC - 1),
                )
            nc.scalar.copy(mt[:, of, :], ps[:])
        mt_g.append(mt)

        # per-slot read-score bias d_s = cr . mem_s -> [128,1] columns
        psd = psA.tile([1, 512], f32, tag="psA")
        for e in range(EC):
            nc.tensor.matmul(
                psd[:], lhsT=crt[:, e : e + 1], rhs=mem[g][:, e, :],
                start=(e == 0), stop=(e == EC - 1),
            )
        sd = ssp.tile([1, 512], f16, tag="sd")
        nc.scalar.copy(sd[:], psd[:])
        psdT = psT.tile([P, 4], f32, tag="psT")
        for bi in range(GB):
            nc.tensor.matmul(
                psdT[:, bi : bi + 1],
                lhsT=sd[0:1, bi * 128 : (bi + 1) * 128],
                rhs=ones_r16[0:1, 0:1],
            )
        dcol = tp.tile([P, 4], f32, tag=f"dcol{g}")
        nc.scalar.copy(dcol[:], psdT[:])
        dcol_g.append(dcol)

        # mem transposed to slot-major for the A_r @ mem matmuls
        mT = mtp.tile([P, 4, E], f16, tag=f"mT{g}")
        for bi in range(GB):
            for e in range(EC):
                pt = psT.tile([P, P], f16, tag="psT")
                nc.tensor.transpose(pt[:], mem[g][:, e, bi * 128 : (bi + 1) * 128], idy[:])
                if e % 2 == 0:
                    nc.vector.tensor_copy(mT[:, bi, e * 128 : (e + 1) * 128], pt[:])
                else:
                    nc.scalar.copy(mT[:, bi, e * 128 : (e + 1) * 128], pt[:])
        mT_g.append(mT)

    for g in range(NG):
        # wave: scores for the whole group first
        eS_l = []
        for bi in range(GB):
            b = g * GB + bi
            ps_s = psA.tile([P, L], f32, tag="psA")
            for of in range(EC):
                nc.tensor.matmul(
                    ps_s[:],
                    lhsT=mt_g[g][:, of, bi * 128 : (bi + 1) * 128],
                    rhs=zt[b][:, of, :],
                    start=(of == 0), stop=(of == EC - 1),
                )
            eS = ewp.tile([P, L], f16, tag="esw")
            nc.scalar.activation(eS[:], ps_s[:], AF.Exp, bias=dcol_g[g][:, bi : bi + 1])
            eS_l.append(eS)

        for bi in range(GB):
            b = g * GB + bi
            eS = eS_l[bi]
            # per-token 1/colsum via ones-matmul + tiny transposes
            psc = psA.tile([1, L], f32, tag="psA")
            nc.tensor.matmul(psc[:], lhsT=ones_c16[:], rhs=eS[:])
            sc = ssp.tile([1, L], f16, tag="sd")
            nc.scalar.copy(sc[:], psc[:])
            psrT = psT.tile([P, 4], f32, tag="psT")
            for t4 in range(4):
                nc.tensor.matmul(
                    psrT[:, t4 : t4 + 1],
                    lhsT=sc[0:1, t4 * 128 : (t4 + 1) * 128],
                    rhs=ones_r16[0:1, 0:1],
                )
            rc4 = tp.tile([P, 4], f32, tag="rc4")
            nc.vector.reciprocal(rc4[:], psrT[:])
            # O_r^T = mem^T @ A^T on unnormalized exp scores
            orr = bap.tile([P, EC, L], f16, tag="ba")
            for c in range(EC):
                pso = psB.tile([P, L], f32, tag="psB")
                nc.tensor.matmul(
                    pso[:], lhsT=mT_g[g][:, bi, c * 128 : (c + 1) * 128], rhs=eS[:]
                )
                nc.vector.tensor_copy(orr[:, c, :], pso[:])
            # out = O_r @ Wvo^T * (1/colsum) + b_out, token-major, fp16 store
            for t4 in range(4):
                osb = op.tile([P, E], f16, tag="osb")
                for n0, nw in ((0, 512), (512, 256)):
                    ps = psA.tile([P, nw], f32, tag="psA")
                    for c in range(EC):
                        nc.tensor.matmul(
                            ps[:],
                            lhsT=orr[:, c, t4 * 128 : (t4 + 1) * 128],
                            rhs=rwo[:, c, n0 : n0 + nw],
                            start=(c == 0), stop=(c == EC - 1),
                        )
                    nc.scalar.activation(
                        osb[:, n0 : n0 + nw], ps[:], AF.Copy,
                        scale=rc4[:, t4 : t4 + 1],
                    )
                    nc.vector.tensor_add(
                        osb[:, n0 : n0 + nw], osb[:, n0 : n0 + nw],
                        brep16[:, n0 : n0 + nw],
                    )
                nc.sync.dma_start(D["out"][b, t4 * 128 : (t4 + 1) * 128, :], osb[:])


def _build():
    if "nc" in _CACHE:
        return _CACHE["nc"]
    nc = bacc.Bacc(
        "TRN2", target_bir_lowering=False, debug=False, enable_asserts=False
    )
    D = {}
    D["z"] = nc.dram_tensor("z", [NB, E, L], f16, kind="ExternalInput").ap()
    D["ztm"] = nc.dram_tensor("ztm", [NB, L, E], f16, kind="ExternalInput").ap()
    for name in ("wq", "whhr", "whhz", "whhn", "rwk", "rwo"):
        D[name] = nc.dram_tensor(name, [E, E], f16, kind="ExternalInput").ap()
    for name in ("wihr8", "wihz8", "wihn8"):
        D[name] = nc.dram_tensor(name, [P, 3, 2, E], f8e5, kind="ExternalInput").ap()
    D["bias"] = nc.dram_tensor("bias", [P, 42], f32, kind="ExternalInput").ap()
    D["brep16"] = nc.dram_tensor("brep16", [P, E], f16, kind="ExternalInput").ap()
    D["crt"] = nc.dram_tensor("crt", [P, EC], f16, kind="ExternalInput").ap()
    D["bgl"] = nc.dram_tensor("bgl", [2, E], f16, kind="ExternalInput").ap()
    D["onesrow"] = nc.dram_tensor("onesrow", [1, 512], f16, kind="ExternalInput").ap()
    D["slots"] = nc.dram_tensor("slots", [E, S], f16, kind="ExternalInput").ap()
    D["out"] = nc.dram_tensor("out", [NB, L, E], f16, kind="ExternalOutput").ap()
    with tile.TileContext(nc) as tc:
        with ExitStack() as ctx:
            _emit(nc, tc, ctx, D)
    nc.compile()
    _CACHE["nc"] = nc
    return nc


def _host_prep(inp):
    sq = 1.0 / np.sqrt(float(E))
    f64 = np.float64

    def t16(a):
        return np.ascontiguousarray(np.asarray(a).T).astype(np.float16)

    def col6(v):
        return np.asarray(v, np.float32).reshape(EC, P).T

    wq_w = np.asarray(inp["w_wq"], f64)
    wk_w = np.asarray(inp["w_wk"], f64)
    wv_w = np.asarray(inp["w_wv"], f64)
    wo_w = np.asarray(inp["w_wo"], f64)
    bq_w = np.asarray(inp["w_bq"], f64)
    bv_w = np.asarray(inp["w_bv"], f64)
    bo_w = np.asarray(inp["w_bo"], f64)
    wih = np.asarray(inp["gru_wih"], f64)
    whh = np.asarray(inp["gru_whh"], f64)
    bih = np.asarray(inp["gru_bih"], f64)
    bhh = np.asarray(inp["gru_bhh"], f64)
    wq_r = np.asarray(inp["r_wq"], f64)
    wk_r = np.asarray(inp["r_wk"], f64)
    wv_r = np.asarray(inp["r_wv"], f64)
    wo_r = np.asarray(inp["r_wo"], f64)
    bq_r = np.asarray(inp["r_bq"], f64)
    bv_r = np.asarray(inp["r_bv"], f64)
    bo_r = np.asarray(inp["r_bo"], f64)

    shared = {}
    # write attention Q/K folded: qtilde = mln @ Aw + cw
    Aw = (sq * wq_w).T @ wk_w                    # [mln-feat, z-feat]
    cw = (sq * bq_w) @ wk_w                      # [z-feat]
    shared["wq"] = np.ascontiguousarray(Aw).astype(np.float16)
    # GRU input side with Wo Wv folded: gi_g = (A@z) @ (Wih_g Wo Wv)^T
    wov = wo_w @ wv_w                            # [E, z-feat]
    bov = wo_w @ bv_w + bo_w                     # [E]
    def dr8(w_gate):
        # hostarr [in, out] -> DoubleRow packed [ki, j, ko, out] fp8
        a = np.ascontiguousarray(np.asarray(w_gate).T).reshape(3, 2, 128, E)
        a = np.ascontiguousarray(a.transpose(2, 0, 1, 3)) * 0.125
        return a.astype(ml_dtypes.float8_e5m2)

    shared["wihr8"] = dr8(wih[0:E] @ wov)
    shared["wihz8"] = dr8(wih[E : 2 * E] @ wov)
    shared["wihn8"] = dr8(wih[2 * E : 3 * E] @ wov)
    shared["whhr"] = t16(whh[0:E])
    shared["whhz"] = t16(whh[E : 2 * E])
    shared["whhn"] = t16(whh[2 * E : 3 * E])
    # read attention: mt = mem @ Ar^T, d = mem . cr, out = O_r @ Wvo^T + bout
    cr = (sq * bq_r) @ wk_r                      # [mem-feat]
    shared["rwk"] = np.ascontiguousarray(wk_r.T @ (sq * wq_r)).astype(np.float16)
    wvo = wo_r @ wv_r                            # [out-feat, mem-feat]
    bout = wo_r @ bv_r + bo_r                    # [out-feat]
    shared["rwo"] = t16(wvo)
    shared["crt"] = np.ascontiguousarray(col6(cr)).astype(np.float16)

    cols = [
        col6(cw),
        col6(wih[0:E] @ bov + bih[0:E] + bhh[0:E]),
        col6(wih[E : 2 * E] @ bov + bih[E : 2 * E] + bhh[E : 2 * E]),
        col6(wih[2 * E : 3 * E] @ bov + bih[2 * E : 3 * E]),
        col6(bhh[2 * E : 3 * E]),
        col6(inp["ln_g"]),
        col6(inp["ln_b"]),
    ]
    shared["bias"] = np.ascontiguousarray(np.concatenate(cols, axis=1), np.float32)
    shared["brep16"] = np.ascontiguousarray(
        np.tile(bout[None, :], (P, 1)).astype(np.float16)
    )
    # bgl rows: [ln_g ; -ln_b] for the K=2 LN broadcast matmul
    shared["bgl"] = np.ascontiguousarray(
        np.stack([np.asarray(inp["ln_g"], f64), -np.asarray(inp["ln_b"], f64)])
    ).astype(np.float16)
    shared["onesrow"] = np.ones((1, 512), np.float16)
    shared["slots"] = t16(np.asarray(inp["slots"])[0])

    z = np.asarray(inp["z"], np.float32)
    zt = np.ascontiguousarray(z.transpose(0, 2, 1)).astype(np.float16)
    in_maps = []
    for c in range(NCORE):
        m = dict(shared)
        m["z"] = np.ascontiguousarray(zt[c * NB : (c + 1) * NB])
        m["ztm"] = np.ascontiguousarray(z[c * NB : (c + 1) * NB].astype(np.float16))
        in_maps.append(m)
    return in_maps


def kernel(**inputs):
    nc = _build()
    in_maps = _host_prep(inputs)
    res = bass_utils.run_bass_kernel_spmd(nc, in_maps, core_ids=list(range(NCORE)))
    out = np.concatenate([res.results[c]["out"] for c in range(NCORE)], axis=0)
    return out.astype(np.float32)
